# revision 5
# baseline (speedup 1.0000x reference)
"""MultiHeadAttention TRN2 kernel: B=2, S=2048, E=1024, H=16, D=64.

Sharding: 8 cores = 2 batches x 4 head-groups (4 heads / 256 channels each).
Each core computes a partial output [2048, 1024] (its heads' contribution to
the final projection); the host sums the 4 partials per batch.

Per-core dataflow (all big matmuls in float32r):
  phase 1: PE-transpose input S-chunks -> xT (E on partitions); project:
           qT/kT [256, S] (bias + 1/8 scaling fused into the PSUM->SBUF
           copy), v [S, 4, 65] with a ones-column (softmax denominator
           comes out of the PV matmul for free).
  phase 2: per (head, sq-half): over 16 sk-chunks: QK^T -> PSUM -> ACT Exp
           -> attnT (f32r) -> PV accumulates [65, 512] PSUMs. Finalize off
           the critical path: drain PV to SBUF, reciprocal of the rowsum,
           PE-broadcast, divide fused into the outT copy (DVE mul reading
           the broadcast PSUM directly).
  phase 3: O-projection + bias -> DMA out.

Tiles are chunked (512-column granularity) so the Tile scheduler can
interleave phases and keep the PE warm (HAM clock gate). PSUM fits in
exactly 8 banks so every pool can stay open for the whole kernel.
"""

import sys

sys.path.insert(0, "/opt/trn_rl_repo")

import numpy as np

B, S, E, H, D = 2, 2048, 1024, 16, 64
HG = 4            # head-groups (cores per batch)
HPG = H // HG     # heads per core = 4
CG = HPG * D      # channels per core = 256
P = 128
NCORES = 8

_CACHED = {}
LAST_RESULT = None


def _build_nc():
    import concourse.bass as bass  # noqa: F401
    import concourse.mybir as mybir
    import concourse.tile as tile
    from concourse import bacc
    from concourse.masks import make_identity

    f32 = mybir.dt.float32
    f32r = mybir.dt.float32r
    AF = mybir.ActivationFunctionType

    nc = bacc.Bacc("TRN2", target_bir_lowering=False, debug=False)

    xq = nc.dram_tensor("xq", [S, E], f32, kind="ExternalInput")
    xk = nc.dram_tensor("xk", [S, E], f32, kind="ExternalInput")
    xv = nc.dram_tensor("xv", [S, E], f32, kind="ExternalInput")
    wq = nc.dram_tensor("wq", [E, CG], f32, kind="ExternalInput")
    wk = nc.dram_tensor("wk", [E, CG], f32, kind="ExternalInput")
    wv = nc.dram_tensor("wv", [E, CG], f32, kind="ExternalInput")
    wo = nc.dram_tensor("wo", [CG, E], f32, kind="ExternalInput")
    bq_d = nc.dram_tensor("bq", [CG], f32, kind="ExternalInput")
    bk_d = nc.dram_tensor("bk", [CG], f32, kind="ExternalInput")
    bv_d = nc.dram_tensor("bv", [CG], f32, kind="ExternalInput")
    bo_d = nc.dram_tensor("bo", [E], f32, kind="ExternalInput")
    out_d = nc.dram_tensor("out", [S, E], f32, kind="ExternalOutput")

    EO = E // P       # 8 e-subtiles
    SC = 4            # S-chunks of 512
    SCW = S // SC     # 512
    S4 = SCW // P     # 4 s-subtiles per chunk
    SO = S // P       # 16 sk-chunks
    MC = CG // P      # 2 m-chunks of head-channels

    with tile.TileContext(nc) as tc:
        with (
            tc.tile_pool(name="pers", bufs=1) as pers,
            tc.tile_pool(name="p1", bufs=2) as p1,
            tc.tile_pool(name="p2", bufs=2) as p2,
            tc.tile_pool(name="px", bufs=1) as px,
            tc.tile_pool(name="pat", bufs=3) as pat,
            tc.tile_pool(name="p3", bufs=2) as p3,
            # PSUM: "w" 2x[128,512]=2 banks, "qk" 2x[128,1024]=4, "pv" 2x[65,512]=2
            tc.tile_pool(name="psw", bufs=2, space="PSUM") as psw,
            tc.tile_pool(name="psqk", bufs=2, space="PSUM") as psqk,
            tc.tile_pool(name="pspv", bufs=2, space="PSUM") as pspv,
        ):
            # ---- static tiles
            wq_sb = pers.tile([P, EO, CG], f32r)
            wk_sb = pers.tile([P, EO, CG], f32r)
            wv_sb = pers.tile([P, EO, CG], f32r)
            wo_sb = pers.tile([P, MC, E], f32r)
            nc.sync.dma_start(wq_sb[:], wq.rearrange("(ko ki) c -> ki ko c", ki=P).bitcast(f32r))
            nc.sync.dma_start(wk_sb[:], wk.rearrange("(ko ki) c -> ki ko c", ki=P).bitcast(f32r))
            nc.sync.dma_start(wv_sb[:], wv.rearrange("(ko ki) c -> ki ko c", ki=P).bitcast(f32r))
            nc.sync.dma_start(wo_sb[:], wo.rearrange("(ho hi) e -> hi ho e", hi=P).bitcast(f32r))

            bq_sb = pers.tile([P, MC], f32)
            bk_sb = pers.tile([P, MC], f32)
            nc.sync.dma_start(bq_sb[:], bq_d.rearrange("(mc p) -> p mc", p=P))
            nc.sync.dma_start(bk_sb[:], bk_d.rearrange("(mc p) -> p mc", p=P))
            bq8_sb = pers.tile([P, MC], f32)
            nc.vector.tensor_scalar_mul(bq8_sb[:], bq_sb[:], 0.125)

            bv_row = pers.tile([1, CG], f32)
            bo_row = pers.tile([1, E], f32)
            nc.sync.dma_start(bv_row[:], bv_d[None, :])
            nc.sync.dma_start(bo_row[:], bo_d[None, :])

            ident32 = pers.tile([P, P], f32)
            make_identity(nc, ident32[:])
            identr = pers.tile([P, P], f32r)
            nc.vector.tensor_copy(identr[:], ident32[:])

            ones_row = pers.tile([1, P], f32)
            nc.gpsimd.memset(ones_row[:], 1.0)
            ones_col = pers.tile([P, 1], f32)
            nc.gpsimd.memset(ones_col[:], 1.0)

            # broadcast bv/bo across partitions (via K=1 matmuls)
            bv_bc = pers.tile([P, CG], f32)
            bo_bc = pers.tile([P, E], f32)
            pb1 = psqk.tile([P, 1024], f32, tag="qk", name="pb1")
            nc.tensor.matmul(pb1[:, :CG], ones_row[:], bv_row[:], start=True, stop=True)
            nc.vector.tensor_copy(bv_bc[:], pb1[:, :CG])
            pb2 = psqk.tile([P, 1024], f32, tag="qk", name="pb2")
            for i in range(2):
                nc.tensor.matmul(pb2[:, i * 512:(i + 1) * 512], ones_row[:],
                                 bo_row[:, i * 512:(i + 1) * 512], start=True, stop=True)
            nc.vector.tensor_copy(bo_bc[:], pb2[:])

            # persistent activations, chunked for fine-grained deps
            qTc = [[pers.tile([P, SCW], f32r, tag=f"qT{m}_{s}", name=f"qT{m}_{s}")
                    for s in range(SC)] for m in range(MC)]
            kTc = [[pers.tile([P, SCW], f32r, tag=f"kT{m}_{s}", name=f"kT{m}_{s}")
                    for s in range(SC)] for m in range(MC)]
            v_c = [pers.tile([P, S4, HPG, D + 1], f32r, tag=f"v{g}", name=f"v{g}")
                   for g in range(SC)]
            outTc = [[pers.tile([P, 1024], f32r, tag=f"oT{m}_{h}", name=f"oT{m}_{h}")
                      for h in range(2)] for m in range(MC)]

            for g in range(SC):
                for s4 in range(S4):
                    nc.vector.tensor_copy(
                        v_c[g][:, s4, :, D:D + 1],
                        ones_col[:, None, :].to_broadcast((P, HPG, 1)),
                    )

            # ============ Phase 1: transpose + QKV projections ============
            for which, src in (("k", xk), ("q", xq), ("v", xv)):
                for sc in range(SC):
                    x_sb = p1.tile([P, S4, E], f32r, tag="xin", name=f"x_{which}{sc}")
                    nc.sync.dma_start(
                        x_sb[:],
                        src[sc * SCW:(sc + 1) * SCW]
                        .rearrange("(s4 si) e -> si s4 e", si=P)
                        .bitcast(f32r),
                    )
                    xt = p1.tile([P, EO, SCW], f32r, tag="xt", name=f"xt_{which}{sc}")
                    for eo in range(EO):
                        pt = psw.tile([P, SCW], f32, tag="w", name=f"pt{which}{sc}_{eo}")
                        for s4 in range(S4):
                            nc.tensor.transpose(
                                pt.bitcast(f32r)[:, s4 * P:(s4 + 1) * P],
                                x_sb[:, s4, eo * P:(eo + 1) * P],
                                identr[:],
                            )
                        nc.vector.tensor_copy(xt[:, eo, :], pt[:])

                    if which in ("q", "k"):
                        w_sb = wq_sb if which == "q" else wk_sb
                        dstT = qTc if which == "q" else kTc
                        bias = bq8_sb if which == "q" else bk_sb
                        scl = 0.125 if which == "q" else 1.0
                        for mc in range(MC):
                            pp = psw.tile([P, SCW], f32, tag="w", name=f"pp{which}{sc}_{mc}")
                            for eo in range(EO):
                                nc.tensor.matmul(
                                    pp[:],
                                    w_sb[:, eo, mc * P:(mc + 1) * P],
                                    xt[:, eo, :],
                                    start=(eo == 0),
                                    stop=(eo == EO - 1),
                                )
                            nc.scalar.activation(
                                dstT[mc][sc][:],
                                pp[:],
                                AF.Identity,
                                bias=bias[:, mc:mc + 1],
                                scale=scl,
                            )
                    else:  # v: natural layout [s, channels]
                        for s4 in range(S4):
                            pv = psw.tile([P, SCW], f32, tag="w", name=f"ppv{sc}_{s4}")
                            for eo in range(EO):
                                nc.tensor.matmul(
                                    pv[:, :CG],
                                    xt[:, eo, s4 * P:(s4 + 1) * P],
                                    wv_sb[:, eo, :],
                                    start=(eo == 0),
                                    stop=(eo == EO - 1),
                                )
                            nc.vector.tensor_add(
                                v_c[sc][:, s4, :, 0:D],
                                pv[:, :CG].rearrange("p (h d) -> p h d", h=HPG),
                                bv_bc.rearrange("p (h d) -> p h d", h=HPG),
                            )

            # ============ Phase 2: attention per (head, sq-half) ============
            for h in range(HPG):
                mcq = h // 2
                off = (h % 2) * D
                for half in range(2):
                    pv_ps = [pspv.tile([D + 1, 512], f32, tag="pv", name=f"pv{h}_{half}_{i}")
                             for i in range(2)]
                    for so in range(SO):
                        qk = psqk.tile([P, 1024], f32, tag="qk", name=f"qk{h}_{half}_{so}")
                        for sq2 in range(2):
                            nc.tensor.matmul(
                                qk[:, sq2 * 512:(sq2 + 1) * 512],
                                kTc[mcq][so // S4][off:off + D, (so % S4) * P:(so % S4 + 1) * P],
                                qTc[mcq][half * 2 + sq2][off:off + D, :],
                                start=True,
                                stop=True,
                            )
                        at = pat.tile([P, 1024], f32r, tag="at", name=f"at{h}_{half}_{so}")
                        nc.scalar.activation(at[:], qk[:], AF.Exp, scale=1.0)
                        for sq2 in range(2):
                            nc.tensor.matmul(
                                pv_ps[sq2][:],
                                v_c[so // S4][:, so % S4, h, :],
                                at[:, sq2 * 512:(sq2 + 1) * 512],
                                start=(so == 0),
                                stop=(so == SO - 1),
                            )
                    # ---- finalize (off the QK/exp/PV critical path)
                    oT_tmp = p2.tile([D + 1, 1024], f32, tag="ot", name=f"ot{h}_{half}")
                    for sq2 in range(2):
                        nc.scalar.activation(
                            oT_tmp[:, sq2 * 512:(sq2 + 1) * 512],
                            pv_ps[sq2][:],
                            AF.Identity,
                            scale=1.0,
                        )
                    rr = px.tile([1, 1024], f32, tag="rr", name=f"rr{h}_{half}")
                    nc.vector.reciprocal(rr[:], oT_tmp[D:D + 1, :])
                    bc = psqk.tile([P, 1024], f32, tag="qk", name=f"bc{h}_{half}")
                    for sq2 in range(2):
                        nc.tensor.matmul(
                            bc[:D, sq2 * 512:(sq2 + 1) * 512],
                            ones_row[:, :D],
                            rr[:, sq2 * 512:(sq2 + 1) * 512],
                            start=True,
                            stop=True,
                        )
                    nc.vector.tensor_mul(
                        outTc[mcq][half][off:off + D, :],
                        oT_tmp[0:D, :],
                        bc[0:D, :],
                    )

            # ============ Phase 3: output projection ============
            out_r = out_d.rearrange("(so si) e -> so si e", si=P)
            for so in range(SO):
                po = psqk.tile([P, 1024], f32, tag="qk", name=f"po{so}")
                for ec in range(2):
                    for ho in range(MC):
                        nc.tensor.matmul(
                            po[:, ec * 512:(ec + 1) * 512],
                            outTc[ho][so // 8][:, (so % 8) * P:(so % 8 + 1) * P],
                            wo_sb[:, ho, ec * 512:(ec + 1) * 512],
                            start=(ho == 0),
                            stop=(ho == MC - 1),
                        )
                o_sb = p3.tile([P, E], f32, tag="osb", name=f"osb{so}")
                nc.vector.tensor_add(o_sb[:], po[:], bo_bc[:])
                nc.sync.dma_start(out_r[so], o_sb[:])

    nc.compile()
    return nc


def kernel(query, key, value, Wq, bq, Wk, bk, Wv, bv, Wo, bo):
    global LAST_RESULT
    from concourse.bass_utils import run_bass_kernel_spmd

    if "nc" not in _CACHED:
        _CACHED["nc"] = _build_nc()
    nc = _CACHED["nc"]

    query = np.ascontiguousarray(np.asarray(query, dtype=np.float32))
    key = np.ascontiguousarray(np.asarray(key, dtype=np.float32))
    value = np.ascontiguousarray(np.asarray(value, dtype=np.float32))
    Wq = np.asarray(Wq, dtype=np.float32)
    Wk = np.asarray(Wk, dtype=np.float32)
    Wv = np.asarray(Wv, dtype=np.float32)
    Wo = np.asarray(Wo, dtype=np.float32)
    bq = np.asarray(bq, dtype=np.float32)
    bk = np.asarray(bk, dtype=np.float32)
    bv = np.asarray(bv, dtype=np.float32)
    bo = np.asarray(bo, dtype=np.float32)

    in_maps = []
    for c in range(NCORES):
        b = c // HG
        g = c % HG
        cs = slice(g * CG, (g + 1) * CG)
        in_maps.append({
            "xq": query[b],
            "xk": key[b],
            "xv": value[b],
            "wq": np.ascontiguousarray(Wq[:, cs]),
            "wk": np.ascontiguousarray(Wk[:, cs]),
            "wv": np.ascontiguousarray(Wv[:, cs]),
            "wo": np.ascontiguousarray(Wo[cs, :]),
            "bq": np.ascontiguousarray(bq[cs]),
            "bk": np.ascontiguousarray(bk[cs]),
            "bv": np.ascontiguousarray(bv[cs]),
            "bo": bo,
        })

    res = run_bass_kernel_spmd(nc, in_maps, list(range(NCORES)))
    LAST_RESULT = res

    out = np.empty((B, S, E), dtype=np.float32)
    for b in range(B):
        acc = np.zeros((S, E), dtype=np.float64)
        for g in range(HG):
            acc += res.results[b * HG + g]["out"].astype(np.float64)
        out[b] = acc.astype(np.float32)
    return out


# revision 7
# speedup vs baseline: 1.2468x; 1.2468x over previous
"""MultiHeadAttention TRN2 kernel: B=2, S=2048, E=1024, H=16, D=64.

Sharding: 8 cores = 2 batches x 4 head-groups (4 heads / 256 channels each).
Each core computes a partial output [2048, 1024] (its heads' contribution to
the final projection); the host sums the 4 partials per batch.

Per-core dataflow (all big matmuls in float32r):
  phase 1: PE-transpose input S-chunks -> xT (E on partitions); project:
           qT/kT [256, S] (bias + 1/8 scaling fused into the PSUM->SBUF
           copy), v [S, 4, 65] with a ones-column (softmax denominator
           comes out of the PV matmul for free).
  phase 2: per (head, sq-half): over 16 sk-chunks: QK^T -> PSUM -> ACT Exp
           -> attnT (f32r) -> PV accumulates [65, 512] PSUMs. Finalize off
           the critical path: drain PV to SBUF, reciprocal of the rowsum,
           PE-broadcast, divide fused into the outT copy (DVE mul reading
           the broadcast PSUM directly).
  phase 3: O-projection + bias -> DMA out.

Tiles are chunked (512-column granularity) so the Tile scheduler can
interleave phases and keep the PE warm (HAM clock gate). PSUM fits in
exactly 8 banks so every pool can stay open for the whole kernel.
"""

import sys

sys.path.insert(0, "/opt/trn_rl_repo")

import numpy as np

B, S, E, H, D = 2, 2048, 1024, 16, 64
HG = 4            # head-groups (cores per batch)
HPG = H // HG     # heads per core = 4
CG = HPG * D      # channels per core = 256
P = 128
NCORES = 8

_CACHED = {}
LAST_RESULT = None


def _build_nc():
    import concourse.bass as bass  # noqa: F401
    import concourse.mybir as mybir
    import concourse.tile as tile
    from concourse import bacc
    from concourse.masks import make_identity

    f32 = mybir.dt.float32
    f32r = mybir.dt.float32r
    AF = mybir.ActivationFunctionType

    nc = bacc.Bacc("TRN2", target_bir_lowering=False, debug=False)

    xq = nc.dram_tensor("xq", [S, E], f32, kind="ExternalInput")
    xk = nc.dram_tensor("xk", [S, E], f32, kind="ExternalInput")
    xv = nc.dram_tensor("xv", [S, E], f32, kind="ExternalInput")
    wq = nc.dram_tensor("wq", [E, CG], f32, kind="ExternalInput")
    wk = nc.dram_tensor("wk", [E, CG], f32, kind="ExternalInput")
    wv = nc.dram_tensor("wv", [E, CG], f32, kind="ExternalInput")
    wo = nc.dram_tensor("wo", [CG, E], f32, kind="ExternalInput")
    bq_d = nc.dram_tensor("bq", [CG], f32, kind="ExternalInput")
    bk_d = nc.dram_tensor("bk", [CG], f32, kind="ExternalInput")
    bv_d = nc.dram_tensor("bv", [CG], f32, kind="ExternalInput")
    bo_d = nc.dram_tensor("bo", [E], f32, kind="ExternalInput")
    out_d = nc.dram_tensor("out", [S, E], f32, kind="ExternalOutput")

    EO = E // P       # 8 e-subtiles
    SC = 4            # S-chunks of 512
    SCW = S // SC     # 512
    S4 = SCW // P     # 4 s-subtiles per chunk
    SO = S // P       # 16 sk-chunks
    MC = CG // P      # 2 m-chunks of head-channels

    with tile.TileContext(nc) as tc:
        with (
            tc.tile_pool(name="pers", bufs=1) as pers,
            tc.tile_pool(name="p1", bufs=3) as p1,
            tc.tile_pool(name="p1t", bufs=1) as p1t,
            tc.tile_pool(name="p2", bufs=2) as p2,
            tc.tile_pool(name="pat", bufs=2) as pat,
            tc.tile_pool(name="p3", bufs=2) as p3,
            # PSUM: "w" 2x[128,512]=2 banks, "qk" 2x[128,1024]=4, "pv" 2x[128,512]=2
            tc.tile_pool(name="psw", bufs=2, space="PSUM") as psw,
            tc.tile_pool(name="psqk", bufs=2, space="PSUM") as psqk,
            tc.tile_pool(name="pspv", bufs=2, space="PSUM") as pspv,
        ):
            # ---- static tiles
            wq_sb = pers.tile([P, EO, CG], f32r)
            wk_sb = pers.tile([P, EO, CG], f32r)
            wv_sb = pers.tile([P, EO, CG], f32r)
            wo_sb = pers.tile([P, MC, E], f32r)
            nc.sync.dma_start(wq_sb[:], wq.rearrange("(ko ki) c -> ki ko c", ki=P).bitcast(f32r))
            nc.sync.dma_start(wk_sb[:], wk.rearrange("(ko ki) c -> ki ko c", ki=P).bitcast(f32r))
            nc.sync.dma_start(wv_sb[:], wv.rearrange("(ko ki) c -> ki ko c", ki=P).bitcast(f32r))
            nc.sync.dma_start(wo_sb[:], wo.rearrange("(ho hi) e -> hi ho e", hi=P).bitcast(f32r))

            bq_sb = pers.tile([P, MC], f32)
            bk_sb = pers.tile([P, MC], f32)
            nc.sync.dma_start(bq_sb[:], bq_d.rearrange("(mc p) -> p mc", p=P))
            nc.sync.dma_start(bk_sb[:], bk_d.rearrange("(mc p) -> p mc", p=P))
            bq8_sb = pers.tile([P, MC], f32)
            nc.vector.tensor_scalar_mul(bq8_sb[:], bq_sb[:], 0.125)

            bv_row = pers.tile([1, CG], f32)
            bo_row = pers.tile([1, E], f32)
            nc.sync.dma_start(bv_row[:], bv_d[None, :])
            nc.sync.dma_start(bo_row[:], bo_d[None, :])

            ident32 = pers.tile([P, P], f32)
            make_identity(nc, ident32[:])
            identr = pers.tile([P, P], f32r)
            nc.vector.tensor_copy(identr[:], ident32[:])

            ones_row = pers.tile([1, P], f32)
            nc.gpsimd.memset(ones_row[:], 1.0)
            ones_col = pers.tile([P, 1], f32)
            nc.gpsimd.memset(ones_col[:], 1.0)
            zero_col = pers.tile([P, 1], f32)
            nc.gpsimd.memset(zero_col[:], 0.0)
            ones_mat = pers.tile([P, D], f32)
            nc.gpsimd.memset(ones_mat[:], 1.0)

            # broadcast bv/bo across partitions (via K=1 matmuls)
            bv_bc = pers.tile([P, CG], f32)
            bo_bc = pers.tile([P, E], f32)
            pb1 = psqk.tile([P, 1024], f32, tag="qk", name="pb1")
            nc.tensor.matmul(pb1[:, :CG], ones_row[:], bv_row[:], start=True, stop=True)
            nc.vector.tensor_copy(bv_bc[:], pb1[:, :CG])
            pb2 = psqk.tile([P, 1024], f32, tag="qk", name="pb2")
            for i in range(2):
                nc.tensor.matmul(pb2[:, i * 512:(i + 1) * 512], ones_row[:],
                                 bo_row[:, i * 512:(i + 1) * 512], start=True, stop=True)
            nc.vector.tensor_copy(bo_bc[:], pb2[:])

            # persistent activations, chunked for fine-grained deps
            qTc = [[pers.tile([P, SCW], f32r, tag=f"qT{m}_{s}", name=f"qT{m}_{s}")
                    for s in range(SC)] for m in range(MC)]
            # kT per head, zero-padded to K=128 (the other head's q rows hit
            # zeros, so full-array QK matmuls stay exact)
            kTp = [pers.tile([P, S], f32r, tag=f"kTp{h}", name=f"kTp{h}")
                   for h in range(HPG)]
            VW = HPG * (D + 1) + (P - D - 1)   # 4*65 + 63 = 323
            v_c = [pers.tile([P, S4, VW], f32r, tag=f"v{g}", name=f"v{g}")
                   for g in range(SC)]
            outTc = [[pers.tile([P, 1024], f32r, tag=f"oT{m}_{h}", name=f"oT{m}_{h}")
                      for h in range(2)] for m in range(MC)]

            for h in range(HPG):
                zoff = D if h % 2 == 0 else 0
                nc.vector.tensor_copy(
                    kTp[h][zoff:zoff + D, :],
                    zero_col[:D, :].to_broadcast((D, S)),
                )
            for g in range(SC):
                for s4 in range(S4):
                    nc.vector.tensor_copy(
                        v_c[g][:, s4, 0:HPG * (D + 1)].rearrange("p (h x) -> p h x", h=HPG)[:, :, D:D + 1],
                        ones_col[:, None, :].to_broadcast((P, HPG, 1)),
                    )
                nc.vector.tensor_copy(
                    v_c[g][:, :, HPG * (D + 1):VW],
                    zero_col[:, None, :].to_broadcast((P, S4, P - D - 1)),
                )

            # ============ Phase 1: transpose + QKV projections ============
            for which, src in (("k", xk), ("q", xq), ("v", xv)):
                for sc in range(SC):
                    x_half = []
                    for hf in range(2):
                        xh = p1.tile([P, 2, E], f32r, tag="xin", name=f"x_{which}{sc}_{hf}")
                        nc.sync.dma_start(
                            xh[:],
                            src[sc * SCW + hf * 256:sc * SCW + (hf + 1) * 256]
                            .rearrange("(s4 si) e -> si s4 e", si=P)
                            .bitcast(f32r),
                        )
                        x_half.append(xh)
                    xt = p1t.tile([P, EO, SCW], f32r, tag="xt", name=f"xt_{which}{sc}")
                    for eo in range(EO):
                        pt = psw.tile([P, SCW], f32, tag="w", name=f"pt{which}{sc}_{eo}")
                        for s4 in range(S4):
                            nc.tensor.transpose(
                                pt.bitcast(f32r)[:, s4 * P:(s4 + 1) * P],
                                x_half[s4 // 2][:, s4 % 2, eo * P:(eo + 1) * P],
                                identr[:],
                            )
                        nc.vector.tensor_copy(xt[:, eo, :], pt[:])

                    if which in ("q", "k"):
                        w_sb = wq_sb if which == "q" else wk_sb
                        bias = bq8_sb if which == "q" else bk_sb
                        scl = 0.125 if which == "q" else 1.0
                        for mc in range(MC):
                            pp = psw.tile([P, SCW], f32, tag="w", name=f"pp{which}{sc}_{mc}")
                            for eo in range(EO):
                                nc.tensor.matmul(
                                    pp[:],
                                    w_sb[:, eo, mc * P:(mc + 1) * P],
                                    xt[:, eo, :],
                                    start=(eo == 0),
                                    stop=(eo == EO - 1),
                                )
                            if which == "q":
                                nc.scalar.activation(
                                    qTc[mc][sc][:],
                                    pp[:],
                                    AF.Identity,
                                    bias=bias[:, mc:mc + 1],
                                    scale=scl,
                                )
                            else:
                                for hh in range(2):
                                    h = 2 * mc + hh
                                    nc.scalar.activation(
                                        kTp[h][hh * D:(hh + 1) * D, sc * SCW:(sc + 1) * SCW],
                                        pp[hh * D:(hh + 1) * D, :],
                                        AF.Identity,
                                        bias=bias[hh * D:(hh + 1) * D, mc:mc + 1],
                                        scale=scl,
                                    )
                    else:  # v: natural layout [s, channels]
                        for s4 in range(S4):
                            pv = psw.tile([P, SCW], f32, tag="w", name=f"ppv{sc}_{s4}")
                            for eo in range(EO):
                                nc.tensor.matmul(
                                    pv[:, :CG],
                                    xt[:, eo, s4 * P:(s4 + 1) * P],
                                    wv_sb[:, eo, :],
                                    start=(eo == 0),
                                    stop=(eo == EO - 1),
                                )
                            nc.vector.tensor_add(
                                v_c[sc][:, s4, 0:HPG * (D + 1)]
                                .rearrange("p (h x) -> p h x", h=HPG)[:, :, 0:D],
                                pv[:, :CG].rearrange("p (h d) -> p h d", h=HPG),
                                bv_bc.rearrange("p (h d) -> p h d", h=HPG),
                            )

            # ============ Phase 2: attention per (head, sq-half) ============
            for h in range(HPG):
                mcq = h // 2
                off = (h % 2) * D
                for half in range(2):
                    pv_ps = [pspv.tile([P, 512], f32, tag="pv", name=f"pv{h}_{half}_{i}")
                             for i in range(2)]
                    for so in range(SO):
                        qk = psqk.tile([P, 1024], f32, tag="qk", name=f"qk{h}_{half}_{so}")
                        for sq2 in range(2):
                            nc.tensor.matmul(
                                qk[:, sq2 * 512:(sq2 + 1) * 512],
                                kTp[h][:, so * P:(so + 1) * P],
                                qTc[mcq][half * 2 + sq2][:],
                                start=True,
                                stop=True,
                            )
                        at = pat.tile([P, 1024], f32r, tag="at", name=f"at{h}_{half}_{so}")
                        nc.scalar.activation(at[:], qk[:], AF.Exp, scale=1.0)
                        for sq2 in range(2):
                            nc.tensor.matmul(
                                pv_ps[sq2][:],
                                v_c[so // S4][:, so % S4, h * (D + 1):h * (D + 1) + P],
                                at[:, sq2 * 512:(sq2 + 1) * 512],
                                start=(so == 0),
                                stop=(so == SO - 1),
                            )
                    # ---- finalize (off the QK/exp/PV critical path):
                    # drain rows 0..D of PV psum, broadcast the RAW rowsum via
                    # PE (so the PE never waits on the reciprocal), then
                    # reciprocal + multiply trail on the DVE.
                    oT_tmp = p2.tile([D + 1, 1024], f32, tag="ot", name=f"ot{h}_{half}")
                    for sq2 in range(2):
                        nc.scalar.activation(
                            oT_tmp[:, sq2 * 512:(sq2 + 1) * 512],
                            pv_ps[sq2][0:D + 1, :],
                            AF.Identity,
                            scale=1.0,
                        )
                    bc = psqk.tile([P, 1024], f32, tag="qk", name=f"bc{h}_{half}")
                    for sq2 in range(2):
                        nc.tensor.matmul(
                            bc[:D, sq2 * 512:(sq2 + 1) * 512],
                            ones_mat[D:D + 1, :],
                            oT_tmp[D:D + 1, sq2 * 512:(sq2 + 1) * 512],
                            start=True,
                            stop=True,
                        )
                    rec_sb = p2.tile([D, 1024], f32, tag="rec", name=f"rec{h}_{half}")
                    nc.vector.reciprocal(rec_sb[:], bc[:D, :])
                    nc.vector.tensor_mul(
                        outTc[mcq][half][off:off + D, :],
                        oT_tmp[0:D, :],
                        rec_sb[:],
                    )

            # ============ Phase 3: output projection ============
            out_r = out_d.rearrange("(so si) e -> so si e", si=P)
            for so in range(SO):
                po = psqk.tile([P, 1024], f32, tag="qk", name=f"po{so}")
                for ec in range(2):
                    for ho in range(MC):
                        nc.tensor.matmul(
                            po[:, ec * 512:(ec + 1) * 512],
                            outTc[ho][so // 8][:, (so % 8) * P:(so % 8 + 1) * P],
                            wo_sb[:, ho, ec * 512:(ec + 1) * 512],
                            start=(ho == 0),
                            stop=(ho == MC - 1),
                        )
                o_sb = p3.tile([P, E], f32, tag="osb", name=f"osb{so}")
                nc.vector.tensor_add(o_sb[:], po[:], bo_bc[:])
                nc.sync.dma_start(out_r[so], o_sb[:])

    nc.compile()
    return nc


def kernel(query, key, value, Wq, bq, Wk, bk, Wv, bv, Wo, bo):
    global LAST_RESULT
    from concourse.bass_utils import run_bass_kernel_spmd

    if "nc" not in _CACHED:
        _CACHED["nc"] = _build_nc()
    nc = _CACHED["nc"]

    query = np.ascontiguousarray(np.asarray(query, dtype=np.float32))
    key = np.ascontiguousarray(np.asarray(key, dtype=np.float32))
    value = np.ascontiguousarray(np.asarray(value, dtype=np.float32))
    Wq = np.asarray(Wq, dtype=np.float32)
    Wk = np.asarray(Wk, dtype=np.float32)
    Wv = np.asarray(Wv, dtype=np.float32)
    Wo = np.asarray(Wo, dtype=np.float32)
    bq = np.asarray(bq, dtype=np.float32)
    bk = np.asarray(bk, dtype=np.float32)
    bv = np.asarray(bv, dtype=np.float32)
    bo = np.asarray(bo, dtype=np.float32)

    in_maps = []
    for c in range(NCORES):
        b = c // HG
        g = c % HG
        cs = slice(g * CG, (g + 1) * CG)
        in_maps.append({
            "xq": query[b],
            "xk": key[b],
            "xv": value[b],
            "wq": np.ascontiguousarray(Wq[:, cs]),
            "wk": np.ascontiguousarray(Wk[:, cs]),
            "wv": np.ascontiguousarray(Wv[:, cs]),
            "wo": np.ascontiguousarray(Wo[cs, :]),
            "bq": np.ascontiguousarray(bq[cs]),
            "bk": np.ascontiguousarray(bk[cs]),
            "bv": np.ascontiguousarray(bv[cs]),
            "bo": bo,
        })

    res = run_bass_kernel_spmd(nc, in_maps, list(range(NCORES)))
    LAST_RESULT = res

    out = np.empty((B, S, E), dtype=np.float32)
    for b in range(B):
        acc = np.zeros((S, E), dtype=np.float64)
        for g in range(HG):
            acc += res.results[b * HG + g]["out"].astype(np.float64)
        out[b] = acc.astype(np.float32)
    return out


# revision 8
# speedup vs baseline: 1.2798x; 1.0265x over previous
"""MultiHeadAttention TRN2 kernel: B=2, S=2048, E=1024, H=16, D=64.

Sharding: 8 cores = 2 batches x 4 head-groups (4 heads / 256 channels each).
Each core computes a partial output [2048, 1024] (its heads' contribution to
the final projection); the host sums the 4 partials per batch.

Per-core dataflow (all big matmuls in float32r):
  phase 1: PE-transpose input S-chunks -> xT (E on partitions); project:
           qT/kT [256, S] (bias + 1/8 scaling fused into the PSUM->SBUF
           copy), v [S, 4, 65] with a ones-column (softmax denominator
           comes out of the PV matmul for free).
  phase 2: per (head, sq-half): over 16 sk-chunks: QK^T -> PSUM -> ACT Exp
           -> attnT (f32r) -> PV accumulates [65, 512] PSUMs. Finalize off
           the critical path: drain PV to SBUF, reciprocal of the rowsum,
           PE-broadcast, divide fused into the outT copy (DVE mul reading
           the broadcast PSUM directly).
  phase 3: O-projection + bias -> DMA out.

Tiles are chunked (512-column granularity) so the Tile scheduler can
interleave phases and keep the PE warm (HAM clock gate). PSUM fits in
exactly 8 banks so every pool can stay open for the whole kernel.
"""

import sys

sys.path.insert(0, "/opt/trn_rl_repo")

import numpy as np

B, S, E, H, D = 2, 2048, 1024, 16, 64
HG = 4            # head-groups (cores per batch)
HPG = H // HG     # heads per core = 4
CG = HPG * D      # channels per core = 256
P = 128
NCORES = 8

_CACHED = {}
LAST_RESULT = None


def _build_nc():
    import concourse.bass as bass  # noqa: F401
    import concourse.mybir as mybir
    import concourse.tile as tile
    from concourse import bacc
    from concourse.masks import make_identity

    f32 = mybir.dt.float32
    f32r = mybir.dt.float32r
    AF = mybir.ActivationFunctionType

    nc = bacc.Bacc("TRN2", target_bir_lowering=False, debug=False)

    xq = nc.dram_tensor("xq", [S, E], f32, kind="ExternalInput")
    xk = nc.dram_tensor("xk", [S, E], f32, kind="ExternalInput")
    xv = nc.dram_tensor("xv", [S, E], f32, kind="ExternalInput")
    wq = nc.dram_tensor("wq", [E, CG], f32, kind="ExternalInput")
    wk = nc.dram_tensor("wk", [E, CG], f32, kind="ExternalInput")
    wv = nc.dram_tensor("wv", [E, CG], f32, kind="ExternalInput")
    wo = nc.dram_tensor("wo", [CG, E], f32, kind="ExternalInput")
    bq_d = nc.dram_tensor("bq", [CG], f32, kind="ExternalInput")
    bk_d = nc.dram_tensor("bk", [CG], f32, kind="ExternalInput")
    bv_d = nc.dram_tensor("bv", [CG], f32, kind="ExternalInput")
    bo_d = nc.dram_tensor("bo", [E], f32, kind="ExternalInput")
    out_d = nc.dram_tensor("out", [S, E], f32, kind="ExternalOutput")

    EO = E // P       # 8 e-subtiles
    SC = 4            # S-chunks of 512
    SCW = S // SC     # 512
    S4 = SCW // P     # 4 s-subtiles per chunk
    SO = S // P       # 16 sk-chunks
    MC = CG // P      # 2 m-chunks of head-channels

    with tile.TileContext(nc) as tc:
        with (
            tc.tile_pool(name="pers", bufs=1) as pers,
            tc.tile_pool(name="p1", bufs=3) as p1,
            tc.tile_pool(name="p1t", bufs=1) as p1t,
            tc.tile_pool(name="p2", bufs=2) as p2,
            tc.tile_pool(name="pat", bufs=3) as pat,
            tc.tile_pool(name="p3", bufs=2) as p3,
            # PSUM: "w" 4x[128,512]=4 banks (phase-1 scratch + phase-2 PV
            # accumulators hand off slots), "qk" 2x[128,1024]=4 banks
            tc.tile_pool(name="psw", bufs=4, space="PSUM") as psw,
            tc.tile_pool(name="psqk", bufs=2, space="PSUM") as psqk,
        ):
            # ---- static tiles
            wq_sb = pers.tile([P, EO, CG], f32r)
            wk_sb = pers.tile([P, EO, CG], f32r)
            wv_sb = pers.tile([P, EO, CG], f32r)
            wo_sb = pers.tile([P, MC, E], f32r)
            nc.sync.dma_start(wq_sb[:], wq.rearrange("(ko ki) c -> ki ko c", ki=P).bitcast(f32r))
            nc.sync.dma_start(wk_sb[:], wk.rearrange("(ko ki) c -> ki ko c", ki=P).bitcast(f32r))
            nc.sync.dma_start(wv_sb[:], wv.rearrange("(ko ki) c -> ki ko c", ki=P).bitcast(f32r))
            nc.sync.dma_start(wo_sb[:], wo.rearrange("(ho hi) e -> hi ho e", hi=P).bitcast(f32r))

            bq_sb = pers.tile([P, MC], f32)
            bk_sb = pers.tile([P, MC], f32)
            nc.sync.dma_start(bq_sb[:], bq_d.rearrange("(mc p) -> p mc", p=P))
            nc.sync.dma_start(bk_sb[:], bk_d.rearrange("(mc p) -> p mc", p=P))
            bq8_sb = pers.tile([P, MC], f32)
            nc.vector.tensor_scalar_mul(bq8_sb[:], bq_sb[:], 0.125)

            bv_row = pers.tile([1, CG], f32)
            bo_row = pers.tile([1, E], f32)
            nc.sync.dma_start(bv_row[:], bv_d[None, :])
            nc.sync.dma_start(bo_row[:], bo_d[None, :])

            ident32 = pers.tile([P, P], f32)
            make_identity(nc, ident32[:])
            identr = pers.tile([P, P], f32r)
            nc.vector.tensor_copy(identr[:], ident32[:])

            ones_row = pers.tile([1, P], f32)
            nc.gpsimd.memset(ones_row[:], 1.0)
            ones_col = pers.tile([P, 1], f32)
            nc.gpsimd.memset(ones_col[:], 1.0)
            zero_col = pers.tile([P, 1], f32)
            nc.gpsimd.memset(zero_col[:], 0.0)
            ones_mat = pers.tile([P, D], f32)
            nc.gpsimd.memset(ones_mat[:], 1.0)

            # broadcast bv/bo across partitions (via K=1 matmuls)
            bv_bc = pers.tile([P, CG], f32)
            bo_bc = pers.tile([P, E], f32)
            pb1 = psqk.tile([P, 1024], f32, tag="qk", name="pb1")
            nc.tensor.matmul(pb1[:, :CG], ones_row[:], bv_row[:], start=True, stop=True)
            nc.vector.tensor_copy(bv_bc[:], pb1[:, :CG])
            pb2 = psqk.tile([P, 1024], f32, tag="qk", name="pb2")
            for i in range(2):
                nc.tensor.matmul(pb2[:, i * 512:(i + 1) * 512], ones_row[:],
                                 bo_row[:, i * 512:(i + 1) * 512], start=True, stop=True)
            nc.vector.tensor_copy(bo_bc[:], pb2[:])

            # persistent activations, chunked for fine-grained deps
            qTc = [[pers.tile([P, SCW], f32r, tag=f"qT{m}_{s}", name=f"qT{m}_{s}")
                    for s in range(SC)] for m in range(MC)]
            # kT per head, zero-padded to K=128 (the other head's q rows hit
            # zeros, so full-array QK matmuls stay exact)
            kTp = [pers.tile([P, S], f32r, tag=f"kTp{h}", name=f"kTp{h}")
                   for h in range(HPG)]
            VW = HPG * (D + 1) + (P - D - 1)   # 4*65 + 63 = 323
            v_c = [pers.tile([P, S4, VW], f32r, tag=f"v{g}", name=f"v{g}")
                   for g in range(SC)]
            outTc = [[pers.tile([P, 1024], f32r, tag=f"oT{m}_{h}", name=f"oT{m}_{h}")
                      for h in range(2)] for m in range(MC)]

            for h in range(HPG):
                zoff = D if h % 2 == 0 else 0
                nc.vector.tensor_copy(
                    kTp[h][zoff:zoff + D, :],
                    zero_col[:D, :].to_broadcast((D, S)),
                )
            for g in range(SC):
                for s4 in range(S4):
                    nc.vector.tensor_copy(
                        v_c[g][:, s4, 0:HPG * (D + 1)].rearrange("p (h x) -> p h x", h=HPG)[:, :, D:D + 1],
                        ones_col[:, None, :].to_broadcast((P, HPG, 1)),
                    )
                nc.vector.tensor_copy(
                    v_c[g][:, :, HPG * (D + 1):VW],
                    zero_col[:, None, :].to_broadcast((P, S4, P - D - 1)),
                )

            # ============ Phase 1: transpose + QKV projections ============
            for which, src in (("k", xk), ("q", xq), ("v", xv)):
                for sc in range(SC):
                    x_half = []
                    for hf in range(2):
                        xh = p1.tile([P, 2, E], f32r, tag="xin", name=f"x_{which}{sc}_{hf}")
                        nc.sync.dma_start(
                            xh[:],
                            src[sc * SCW + hf * 256:sc * SCW + (hf + 1) * 256]
                            .rearrange("(s4 si) e -> si s4 e", si=P)
                            .bitcast(f32r),
                        )
                        x_half.append(xh)
                    xt = p1t.tile([P, EO, SCW], f32r, tag="xt", name=f"xt_{which}{sc}")
                    for eo in range(EO):
                        pt = psw.tile([P, SCW], f32, tag="w", name=f"pt{which}{sc}_{eo}")
                        for s4 in range(S4):
                            nc.tensor.transpose(
                                pt.bitcast(f32r)[:, s4 * P:(s4 + 1) * P],
                                x_half[s4 // 2][:, s4 % 2, eo * P:(eo + 1) * P],
                                identr[:],
                            )
                        nc.vector.tensor_copy(xt[:, eo, :], pt[:])

                    if which in ("q", "k"):
                        w_sb = wq_sb if which == "q" else wk_sb
                        bias = bq8_sb if which == "q" else bk_sb
                        scl = 0.125 if which == "q" else 1.0
                        for mc in range(MC):
                            pp = psw.tile([P, SCW], f32, tag="w", name=f"pp{which}{sc}_{mc}")
                            for eo in range(EO):
                                nc.tensor.matmul(
                                    pp[:],
                                    w_sb[:, eo, mc * P:(mc + 1) * P],
                                    xt[:, eo, :],
                                    start=(eo == 0),
                                    stop=(eo == EO - 1),
                                )
                            if which == "q":
                                nc.scalar.activation(
                                    qTc[mc][sc][:],
                                    pp[:],
                                    AF.Identity,
                                    bias=bias[:, mc:mc + 1],
                                    scale=scl,
                                )
                            else:
                                for hh in range(2):
                                    h = 2 * mc + hh
                                    nc.scalar.activation(
                                        kTp[h][hh * D:(hh + 1) * D, sc * SCW:(sc + 1) * SCW],
                                        pp[hh * D:(hh + 1) * D, :],
                                        AF.Identity,
                                        bias=bias[hh * D:(hh + 1) * D, mc:mc + 1],
                                        scale=scl,
                                    )
                    else:  # v: natural layout [s, channels]
                        for s4 in range(S4):
                            pv = psw.tile([P, SCW], f32, tag="w", name=f"ppv{sc}_{s4}")
                            for eo in range(EO):
                                nc.tensor.matmul(
                                    pv[:, :CG],
                                    xt[:, eo, s4 * P:(s4 + 1) * P],
                                    wv_sb[:, eo, :],
                                    start=(eo == 0),
                                    stop=(eo == EO - 1),
                                )
                            nc.vector.tensor_add(
                                v_c[sc][:, s4, 0:HPG * (D + 1)]
                                .rearrange("p (h x) -> p h x", h=HPG)[:, :, 0:D],
                                pv[:, :CG].rearrange("p (h d) -> p h d", h=HPG),
                                bv_bc.rearrange("p (h d) -> p h d", h=HPG),
                            )

            # ============ Phase 2: attention per (head, sq-half) ============
            # Finalize for segment i is emitted after segment i+1's main loop
            # so the PE never stalls at a segment boundary.
            pending = []

            def finalize(h, half, pv_ps):
                mcq = h // 2
                off = (h % 2) * D
                oT_tmp = p2.tile([D + 1, 1024], f32, tag="ot", name=f"ot{h}_{half}")
                for sq2 in range(2):
                    nc.scalar.activation(
                        oT_tmp[:, sq2 * 512:(sq2 + 1) * 512],
                        pv_ps[sq2][0:D + 1, :],
                        AF.Identity,
                        scale=1.0,
                    )
                bc = psqk.tile([P, 1024], f32, tag="qk", name=f"bc{h}_{half}")
                for sq2 in range(2):
                    nc.tensor.matmul(
                        bc[:D, sq2 * 512:(sq2 + 1) * 512],
                        ones_mat[D:D + 1, :],
                        oT_tmp[D:D + 1, sq2 * 512:(sq2 + 1) * 512],
                        start=True,
                        stop=True,
                    )
                rec_sb = p2.tile([D, 1024], f32, tag="rec", name=f"rec{h}_{half}")
                nc.vector.reciprocal(rec_sb[:], bc[:D, :])
                nc.vector.tensor_mul(
                    outTc[mcq][half][off:off + D, :],
                    oT_tmp[0:D, :],
                    rec_sb[:],
                )

            for h in range(HPG):
                mcq = h // 2
                off = (h % 2) * D
                for half in range(2):
                    pv_ps = [psw.tile([P, 512], f32, tag="w", name=f"pv{h}_{half}_{i}")
                             for i in range(2)]
                    for so in range(SO):
                        qk = psqk.tile([P, 1024], f32, tag="qk", name=f"qk{h}_{half}_{so}")
                        for sq2 in range(2):
                            nc.tensor.matmul(
                                qk[:, sq2 * 512:(sq2 + 1) * 512],
                                kTp[h][:, so * P:(so + 1) * P],
                                qTc[mcq][half * 2 + sq2][:],
                                start=True,
                                stop=True,
                            )
                        at = pat.tile([P, 1024], f32r, tag="at", name=f"at{h}_{half}_{so}")
                        nc.scalar.activation(at[:], qk[:], AF.Exp, scale=1.0)
                        for sq2 in range(2):
                            nc.tensor.matmul(
                                pv_ps[sq2][:],
                                v_c[so // S4][:, so % S4, h * (D + 1):h * (D + 1) + P],
                                at[:, sq2 * 512:(sq2 + 1) * 512],
                                start=(so == 0),
                                stop=(so == SO - 1),
                            )
                    pending.append((h, half, pv_ps))
                    if len(pending) > 1:
                        finalize(*pending.pop(0))
            while pending:
                finalize(*pending.pop(0))

            # ============ Phase 3: output projection ============
            out_r = out_d.rearrange("(so si) e -> so si e", si=P)
            for so in range(SO):
                po = psqk.tile([P, 1024], f32, tag="qk", name=f"po{so}")
                for ec in range(2):
                    for ho in range(MC):
                        nc.tensor.matmul(
                            po[:, ec * 512:(ec + 1) * 512],
                            outTc[ho][so // 8][:, (so % 8) * P:(so % 8 + 1) * P],
                            wo_sb[:, ho, ec * 512:(ec + 1) * 512],
                            start=(ho == 0),
                            stop=(ho == MC - 1),
                        )
                o_sb = p3.tile([P, E], f32, tag="osb", name=f"osb{so}")
                nc.vector.tensor_add(o_sb[:], po[:], bo_bc[:])
                nc.sync.dma_start(out_r[so], o_sb[:])

    nc.compile()
    return nc


def kernel(query, key, value, Wq, bq, Wk, bk, Wv, bv, Wo, bo):
    global LAST_RESULT
    from concourse.bass_utils import run_bass_kernel_spmd

    if "nc" not in _CACHED:
        _CACHED["nc"] = _build_nc()
    nc = _CACHED["nc"]

    query = np.ascontiguousarray(np.asarray(query, dtype=np.float32))
    key = np.ascontiguousarray(np.asarray(key, dtype=np.float32))
    value = np.ascontiguousarray(np.asarray(value, dtype=np.float32))
    Wq = np.asarray(Wq, dtype=np.float32)
    Wk = np.asarray(Wk, dtype=np.float32)
    Wv = np.asarray(Wv, dtype=np.float32)
    Wo = np.asarray(Wo, dtype=np.float32)
    bq = np.asarray(bq, dtype=np.float32)
    bk = np.asarray(bk, dtype=np.float32)
    bv = np.asarray(bv, dtype=np.float32)
    bo = np.asarray(bo, dtype=np.float32)

    in_maps = []
    for c in range(NCORES):
        b = c // HG
        g = c % HG
        cs = slice(g * CG, (g + 1) * CG)
        in_maps.append({
            "xq": query[b],
            "xk": key[b],
            "xv": value[b],
            "wq": np.ascontiguousarray(Wq[:, cs]),
            "wk": np.ascontiguousarray(Wk[:, cs]),
            "wv": np.ascontiguousarray(Wv[:, cs]),
            "wo": np.ascontiguousarray(Wo[cs, :]),
            "bq": np.ascontiguousarray(bq[cs]),
            "bk": np.ascontiguousarray(bk[cs]),
            "bv": np.ascontiguousarray(bv[cs]),
            "bo": bo,
        })

    res = run_bass_kernel_spmd(nc, in_maps, list(range(NCORES)))
    LAST_RESULT = res

    out = np.empty((B, S, E), dtype=np.float32)
    for b in range(B):
        acc = np.zeros((S, E), dtype=np.float64)
        for g in range(HG):
            acc += res.results[b * HG + g]["out"].astype(np.float64)
        out[b] = acc.astype(np.float32)
    return out


# revision 9
# speedup vs baseline: 1.4580x; 1.1393x over previous
"""MultiHeadAttention TRN2 kernel: B=2, S=2048, E=1024, H=16, D=64.

Sharding: 8 cores = 2 batches x 4 head-groups (4 heads / 256 channels each).
Each core computes a partial output [2048, 1024] (its heads' contribution to
the final projection); the host sums the 4 partials per batch.

Per-core dataflow (all big matmuls in float32r):
  phase 1: PE-transpose input S-chunks -> xT (E on partitions); project:
           qT/kT [256, S] (bias + 1/8 scaling fused into the PSUM->SBUF
           copy), v [S, 4, 65] with a ones-column (softmax denominator
           comes out of the PV matmul for free).
  phase 2: per (head, sq-half): over 16 sk-chunks: QK^T -> PSUM -> ACT Exp
           -> attnT (f32r) -> PV accumulates [65, 512] PSUMs. Finalize off
           the critical path: drain PV to SBUF, reciprocal of the rowsum,
           PE-broadcast, divide fused into the outT copy (DVE mul reading
           the broadcast PSUM directly).
  phase 3: O-projection + bias -> DMA out.

Tiles are chunked (512-column granularity) so the Tile scheduler can
interleave phases and keep the PE warm (HAM clock gate). PSUM fits in
exactly 8 banks so every pool can stay open for the whole kernel.
"""

import sys

sys.path.insert(0, "/opt/trn_rl_repo")

import numpy as np

B, S, E, H, D = 2, 2048, 1024, 16, 64
HG = 4            # head-groups (cores per batch)
HPG = H // HG     # heads per core = 4
CG = HPG * D      # channels per core = 256
P = 128
NCORES = 8

_CACHED = {}
LAST_RESULT = None


def _build_nc():
    import concourse.bass as bass  # noqa: F401
    import concourse.mybir as mybir
    import concourse.tile as tile
    from concourse import bacc
    from concourse.masks import make_identity

    f32 = mybir.dt.float32
    f32r = mybir.dt.float32r
    AF = mybir.ActivationFunctionType

    nc = bacc.Bacc("TRN2", target_bir_lowering=False, debug=False)

    xq = nc.dram_tensor("xq", [S, E], f32, kind="ExternalInput")
    xk = nc.dram_tensor("xk", [S, E], f32, kind="ExternalInput")
    xv = nc.dram_tensor("xv", [S, E], f32, kind="ExternalInput")
    wq = nc.dram_tensor("wq", [E, CG], f32, kind="ExternalInput")
    wk = nc.dram_tensor("wk", [E, CG], f32, kind="ExternalInput")
    wv = nc.dram_tensor("wv", [E, CG], f32, kind="ExternalInput")
    wo = nc.dram_tensor("wo", [CG, E], f32, kind="ExternalInput")
    bq_d = nc.dram_tensor("bq", [CG], f32, kind="ExternalInput")
    bk_d = nc.dram_tensor("bk", [CG], f32, kind="ExternalInput")
    bv_d = nc.dram_tensor("bv", [CG], f32, kind="ExternalInput")
    bo_d = nc.dram_tensor("bo", [E], f32, kind="ExternalInput")
    out_d = nc.dram_tensor("out", [S, E], f32, kind="ExternalOutput")

    EO = E // P       # 8 e-subtiles
    SC = 4            # S-chunks of 512
    SCW = S // SC     # 512
    S4 = SCW // P     # 4 s-subtiles per chunk
    SO = S // P       # 16 sk-chunks
    MC = CG // P      # 2 m-chunks of head-channels

    with tile.TileContext(nc) as tc:
        with (
            tc.tile_pool(name="pers", bufs=1) as pers,
            tc.tile_pool(name="p1", bufs=3) as p1,
            tc.tile_pool(name="p1t", bufs=1) as p1t,
            tc.tile_pool(name="p2", bufs=2) as p2,
            tc.tile_pool(name="pat", bufs=3) as pat,
            tc.tile_pool(name="p3", bufs=2) as p3,
            # PSUM: "w" 4x[128,512]=4 banks (phase-1 scratch + phase-2 PV
            # accumulators hand off slots), "qk" 2x[128,1024]=4 banks
            tc.tile_pool(name="psw", bufs=4, space="PSUM") as psw,
            tc.tile_pool(name="psqk", bufs=2, space="PSUM") as psqk,
        ):
            # ---- static tiles
            wq_sb = pers.tile([P, EO, CG], f32r)
            wk_sb = pers.tile([P, EO, CG], f32r)
            wv_sb = pers.tile([P, EO, CG], f32r)
            wo_sb = pers.tile([P, MC, E], f32r)
            nc.sync.dma_start(wq_sb[:], wq.rearrange("(ko ki) c -> ki ko c", ki=P).bitcast(f32r))
            nc.sync.dma_start(wk_sb[:], wk.rearrange("(ko ki) c -> ki ko c", ki=P).bitcast(f32r))
            nc.sync.dma_start(wv_sb[:], wv.rearrange("(ko ki) c -> ki ko c", ki=P).bitcast(f32r))
            nc.sync.dma_start(wo_sb[:], wo.rearrange("(ho hi) e -> hi ho e", hi=P).bitcast(f32r))

            bq_sb = pers.tile([P, MC], f32)
            bk_sb = pers.tile([P, MC], f32)
            nc.sync.dma_start(bq_sb[:], bq_d.rearrange("(mc p) -> p mc", p=P))
            nc.sync.dma_start(bk_sb[:], bk_d.rearrange("(mc p) -> p mc", p=P))
            bq8_sb = pers.tile([P, MC], f32)
            nc.vector.tensor_scalar_mul(bq8_sb[:], bq_sb[:], 0.125)

            bv_row = pers.tile([1, CG], f32)
            bo_row = pers.tile([1, E], f32)
            nc.sync.dma_start(bv_row[:], bv_d[None, :])
            nc.sync.dma_start(bo_row[:], bo_d[None, :])

            ident32 = pers.tile([P, P], f32)
            make_identity(nc, ident32[:])
            identr = pers.tile([P, P], f32r)
            nc.vector.tensor_copy(identr[:], ident32[:])

            ones_row = pers.tile([1, P], f32)
            nc.gpsimd.memset(ones_row[:], 1.0)
            ones_col = pers.tile([P, 1], f32)
            nc.gpsimd.memset(ones_col[:], 1.0)
            zero_col = pers.tile([P, 1], f32)
            nc.gpsimd.memset(zero_col[:], 0.0)
            ones_mat = pers.tile([P, D], f32)
            nc.gpsimd.memset(ones_mat[:], 1.0)

            # broadcast bv/bo across partitions (via K=1 matmuls)
            bv_bc = pers.tile([P, CG], f32)
            bo_bc = pers.tile([P, E], f32)
            pb1 = psqk.tile([P, 1024], f32, tag="qk", name="pb1")
            nc.tensor.matmul(pb1[:, :CG], ones_row[:], bv_row[:], start=True, stop=True)
            nc.vector.tensor_copy(bv_bc[:], pb1[:, :CG])
            pb2 = psqk.tile([P, 1024], f32, tag="qk", name="pb2")
            for i in range(2):
                nc.tensor.matmul(pb2[:, i * 512:(i + 1) * 512], ones_row[:],
                                 bo_row[:, i * 512:(i + 1) * 512], start=True, stop=True)
            nc.vector.tensor_copy(bo_bc[:], pb2[:])

            # persistent activations, chunked for fine-grained deps
            qTc = [[pers.tile([P, SCW], f32r, tag=f"qT{m}_{s}", name=f"qT{m}_{s}")
                    for s in range(SC)] for m in range(MC)]
            # kT per head, zero-padded to K=128 (the other head's q rows hit
            # zeros, so full-array QK matmuls stay exact)
            kTp = [pers.tile([P, S], f32r, tag=f"kTp{h}", name=f"kTp{h}")
                   for h in range(HPG)]
            VW = HPG * (D + 1) + (P - D - 1)   # 4*65 + 63 = 323
            v_c = [pers.tile([P, S4, VW], f32r, tag=f"v{g}", name=f"v{g}")
                   for g in range(SC)]
            outTc = [[pers.tile([P, 1024], f32r, tag=f"oT{m}_{h}", name=f"oT{m}_{h}")
                      for h in range(2)] for m in range(MC)]

            for h in range(HPG):
                zoff = D if h % 2 == 0 else 0
                nc.vector.tensor_copy(
                    kTp[h][zoff:zoff + D, :],
                    zero_col[:D, :].to_broadcast((D, S)),
                )
            for g in range(SC):
                for s4 in range(S4):
                    nc.vector.tensor_copy(
                        v_c[g][:, s4, 0:HPG * (D + 1)].rearrange("p (h x) -> p h x", h=HPG)[:, :, D:D + 1],
                        ones_col[:, None, :].to_broadcast((P, HPG, 1)),
                    )
                nc.vector.tensor_copy(
                    v_c[g][:, :, HPG * (D + 1):VW],
                    zero_col[:, None, :].to_broadcast((P, S4, P - D - 1)),
                )

            # ============ Phase 1: transpose + QKV projections ============
            for which, src in (("k", xk), ("q", xq), ("v", xv)):
                for sc in range(SC):
                    x_half = []
                    for hf in range(2):
                        xh = p1.tile([P, 2, E], f32r, tag="xin", name=f"x_{which}{sc}_{hf}")
                        nc.sync.dma_start(
                            xh[:],
                            src[sc * SCW + hf * 256:sc * SCW + (hf + 1) * 256]
                            .rearrange("(s4 si) e -> si s4 e", si=P)
                            .bitcast(f32r),
                        )
                        x_half.append(xh)
                    xt = p1t.tile([P, EO, SCW], f32r, tag="xt", name=f"xt_{which}{sc}")
                    for eo in range(EO):
                        pt = psw.tile([P, SCW], f32, tag="w", name=f"pt{which}{sc}_{eo}")
                        for s4 in range(S4):
                            nc.tensor.transpose(
                                pt.bitcast(f32r)[:, s4 * P:(s4 + 1) * P],
                                x_half[s4 // 2][:, s4 % 2, eo * P:(eo + 1) * P],
                                identr[:],
                            )
                        nc.vector.tensor_copy(xt[:, eo, :], pt[:])

                    if which in ("q", "k"):
                        w_sb = wq_sb if which == "q" else wk_sb
                        bias = bq8_sb if which == "q" else bk_sb
                        scl = 0.125 if which == "q" else 1.0
                        for mc in range(MC):
                            pp = psw.tile([P, SCW], f32, tag="w", name=f"pp{which}{sc}_{mc}")
                            for eo in range(EO):
                                nc.tensor.matmul(
                                    pp[:],
                                    w_sb[:, eo, mc * P:(mc + 1) * P],
                                    xt[:, eo, :],
                                    start=(eo == 0),
                                    stop=(eo == EO - 1),
                                )
                            if which == "q":
                                nc.scalar.activation(
                                    qTc[mc][sc][:],
                                    pp[:],
                                    AF.Identity,
                                    bias=bias[:, mc:mc + 1],
                                    scale=scl,
                                )
                            else:
                                for hh in range(2):
                                    h = 2 * mc + hh
                                    nc.scalar.activation(
                                        kTp[h][hh * D:(hh + 1) * D, sc * SCW:(sc + 1) * SCW],
                                        pp[hh * D:(hh + 1) * D, :],
                                        AF.Identity,
                                        bias=bias[hh * D:(hh + 1) * D, mc:mc + 1],
                                        scale=scl,
                                    )
                    else:  # v: natural layout [s, channels]
                        for s4 in range(S4):
                            pv = psw.tile([P, SCW], f32, tag="w", name=f"ppv{sc}_{s4}")
                            for eo in range(EO):
                                nc.tensor.matmul(
                                    pv[:, :CG],
                                    xt[:, eo, s4 * P:(s4 + 1) * P],
                                    wv_sb[:, eo, :],
                                    start=(eo == 0),
                                    stop=(eo == EO - 1),
                                )
                            nc.vector.tensor_add(
                                v_c[sc][:, s4, 0:HPG * (D + 1)]
                                .rearrange("p (h x) -> p h x", h=HPG)[:, :, 0:D],
                                pv[:, :CG].rearrange("p (h d) -> p h d", h=HPG),
                                bv_bc.rearrange("p (h d) -> p h d", h=HPG),
                            )

            # ============ Phase 2: attention per (head, sq-half) ============
            # Finalize of segment i is split and woven into segment i+1:
            # part A (ACT drain of the PV psums + DVE reciprocal of the
            # rowsum) near its start, part B (PE broadcast + DVE multiply,
            # which holds a "qk" psum slot only briefly) at its end.
            state = {}

            def finalize_a(h, half, pv_ps):
                oT_tmp = p2.tile([D + 1, 1024], f32, tag="ot", name=f"ot{h}_{half}")
                for sq2 in range(2):
                    nc.scalar.activation(
                        oT_tmp[:, sq2 * 512:(sq2 + 1) * 512],
                        pv_ps[sq2][0:D + 1, :],
                        AF.Identity,
                        scale=1.0,
                    )
                rr = p2.tile([1, 1024], f32, tag="rr", name=f"rr{h}_{half}")
                nc.vector.reciprocal(rr[:], oT_tmp[D:D + 1, :])
                return oT_tmp, rr

            def finalize_b(h, half, oT_tmp, rr):
                mcq = h // 2
                off = (h % 2) * D
                bc = psqk.tile([P, 1024], f32, tag="qk", name=f"bc{h}_{half}")
                for sq2 in range(2):
                    nc.tensor.matmul(
                        bc[:D, sq2 * 512:(sq2 + 1) * 512],
                        ones_row[:, :D],
                        rr[:, sq2 * 512:(sq2 + 1) * 512],
                        start=True,
                        stop=True,
                    )
                nc.vector.tensor_mul(
                    outTc[mcq][half][off:off + D, :],
                    oT_tmp[0:D, :],
                    bc[0:D, :],
                )

            for h in range(HPG):
                mcq = h // 2
                off = (h % 2) * D
                for half in range(2):
                    pv_ps = [psw.tile([P, 512], f32, tag="w", name=f"pv{h}_{half}_{i}")
                             for i in range(2)]
                    for so in range(SO):
                        qk = psqk.tile([P, 1024], f32, tag="qk", name=f"qk{h}_{half}_{so}")
                        for sq2 in range(2):
                            nc.tensor.matmul(
                                qk[:, sq2 * 512:(sq2 + 1) * 512],
                                kTp[h][:, so * P:(so + 1) * P],
                                qTc[mcq][half * 2 + sq2][:],
                                start=True,
                                stop=True,
                            )
                        at = pat.tile([P, 1024], f32r, tag="at", name=f"at{h}_{half}_{so}")
                        nc.scalar.activation(at[:], qk[:], AF.Exp, scale=1.0)
                        for sq2 in range(2):
                            nc.tensor.matmul(
                                pv_ps[sq2][:],
                                v_c[so // S4][:, so % S4, h * (D + 1):h * (D + 1) + P],
                                at[:, sq2 * 512:(sq2 + 1) * 512],
                                start=(so == 0),
                                stop=(so == SO - 1),
                            )
                        if so == 1 and "pv" in state:
                            ph, phalf, ppv = state.pop("pv")
                            state["ab"] = (ph, phalf, *finalize_a(ph, phalf, ppv))
                    if "ab" in state:
                        finalize_b(*state.pop("ab"))
                    state["pv"] = (h, half, pv_ps)
            ph, phalf, ppv = state.pop("pv")
            finalize_b(ph, phalf, *finalize_a(ph, phalf, ppv))

            # ============ Phase 3: output projection ============
            out_r = out_d.rearrange("(so si) e -> so si e", si=P)
            for so in range(SO):
                po = psqk.tile([P, 1024], f32, tag="qk", name=f"po{so}")
                for ec in range(2):
                    for ho in range(MC):
                        nc.tensor.matmul(
                            po[:, ec * 512:(ec + 1) * 512],
                            outTc[ho][so // 8][:, (so % 8) * P:(so % 8 + 1) * P],
                            wo_sb[:, ho, ec * 512:(ec + 1) * 512],
                            start=(ho == 0),
                            stop=(ho == MC - 1),
                        )
                o_sb = p3.tile([P, E], f32, tag="osb", name=f"osb{so}")
                nc.vector.tensor_add(o_sb[:], po[:], bo_bc[:])
                nc.sync.dma_start(out_r[so], o_sb[:])

    nc.compile()
    return nc


def kernel(query, key, value, Wq, bq, Wk, bk, Wv, bv, Wo, bo):
    global LAST_RESULT
    from concourse.bass_utils import run_bass_kernel_spmd

    if "nc" not in _CACHED:
        _CACHED["nc"] = _build_nc()
    nc = _CACHED["nc"]

    query = np.ascontiguousarray(np.asarray(query, dtype=np.float32))
    key = np.ascontiguousarray(np.asarray(key, dtype=np.float32))
    value = np.ascontiguousarray(np.asarray(value, dtype=np.float32))
    Wq = np.asarray(Wq, dtype=np.float32)
    Wk = np.asarray(Wk, dtype=np.float32)
    Wv = np.asarray(Wv, dtype=np.float32)
    Wo = np.asarray(Wo, dtype=np.float32)
    bq = np.asarray(bq, dtype=np.float32)
    bk = np.asarray(bk, dtype=np.float32)
    bv = np.asarray(bv, dtype=np.float32)
    bo = np.asarray(bo, dtype=np.float32)

    in_maps = []
    for c in range(NCORES):
        b = c // HG
        g = c % HG
        cs = slice(g * CG, (g + 1) * CG)
        in_maps.append({
            "xq": query[b],
            "xk": key[b],
            "xv": value[b],
            "wq": np.ascontiguousarray(Wq[:, cs]),
            "wk": np.ascontiguousarray(Wk[:, cs]),
            "wv": np.ascontiguousarray(Wv[:, cs]),
            "wo": np.ascontiguousarray(Wo[cs, :]),
            "bq": np.ascontiguousarray(bq[cs]),
            "bk": np.ascontiguousarray(bk[cs]),
            "bv": np.ascontiguousarray(bv[cs]),
            "bo": bo,
        })

    res = run_bass_kernel_spmd(nc, in_maps, list(range(NCORES)))
    LAST_RESULT = res

    out = np.empty((B, S, E), dtype=np.float32)
    for b in range(B):
        acc = np.zeros((S, E), dtype=np.float64)
        for g in range(HG):
            acc += res.results[b * HG + g]["out"].astype(np.float64)
        out[b] = acc.astype(np.float32)
    return out
